# revision 12
# baseline (speedup 1.0000x reference)
"""Trainium2 Bass kernel for BiLinearSigmoidAttention.

Reference math (per batch b, with L = length[b]):
    qn = l2norm(query), cn = l2norm(context)
    raw[q,k] = qn[q] . cn[k]            (masked: k >= L -> -1e30)
    sig = sigmoid(raw)
    den[q] = max(sum_k sig[q,k], 1)
    scores[q,k] = sig[q,k] / den[q]     (rows q >= L zeroed)
    att[q,:] = sum_k scores[q,k] * context[k,:]
    out = concat([qn, att], -1)
returns (out [B,S,2D], scores [B,S,S])

Device mapping (8 NeuronCores, pure data parallel over B=32 -> 4 per core).

Engine plan per batch (PE kept dense; the whole PE path is bf16 since
walrus forbids mixing 32-bit with 16-bit matmul operands; rel-err budget
is 2e-2, bf16 lands ~3e-3):
  - q/context are loaded straight into bf16 via SWDGE casting DMAs (no
    fp32 staging in SBUF, no engine cast passes).
  - context transposes start as soon as each s-tile lands; q is
    normalized in place (qb *= 1/||q||) after a single batched
    sqrt/reciprocal, stored to out via a casting DMA, then transposed.
  - mm1: sigT[k,q] = sigmoid(cTb.T @ qTb + keybias), weights reused
    across the two q-halves (kt->dch->qc loop order); context l2-norm
    folded into the sigmoid per-partition scale; evicts to bf16 sg.
  - mm2: att[q,d] = sgblk.T @ cb; denominator rides the same weights
    via a tiny ones matmul.
  - scores out: PE transposes of bf16 sg blocks, scaled by w=qmask/den
    during PSUM->SBUF eviction (split across ACT and DVE).
"""

import numpy as np
import ml_dtypes

import concourse.bacc as bacc
import concourse.mybir as mybir
import concourse.tile as tile
from concourse.bass_utils import run_bass_kernel_spmd

B, S, D = 32, 1024, 512
NCORES = 8
BPC = B // NCORES          # batches per core
P = 128                    # partitions
NT = S // P                # 8 s-tiles
ND = D // P                # 4 d-chunks
NEG = np.float32(-1e30)

F32 = mybir.dt.float32
F32R = mybir.dt.float32r
BF16 = mybir.dt.bfloat16
FP8 = mybir.dt.float8e4
PM = mybir.MatmulPerfMode
AF = mybir.ActivationFunctionType
ALU = mybir.AluOpType
AX = mybir.AxisListType


def build_kernel():
    nc = bacc.Bacc("TRN2", target_bir_lowering=False, debug=False)

    q_d = nc.dram_tensor("query", [BPC, S, D], F32, kind="ExternalInput")
    c_d = nc.dram_tensor("context", [BPC, S, D], F32, kind="ExternalInput")
    # keybias[b, p, kt] = 0 if kt*P+p < L else -1e30
    kb_d = nc.dram_tensor("keybias", [BPC, P, NT], F32, kind="ExternalInput")
    # qmask[b, p, qb] = 1 if qb*P+p < L else 0
    qm_d = nc.dram_tensor("qmask", [BPC, P, NT], F32, kind="ExternalInput")
    id_d = nc.dram_tensor("identb", [P, P], BF16, kind="ExternalInput")
    on_d = nc.dram_tensor("onesb", [P, 2], BF16, kind="ExternalInput")
    out_d = nc.dram_tensor("out", [BPC, S, 2 * D], F32, kind="ExternalOutput")
    sc_d = nc.dram_tensor("scores", [BPC, S, S], F32, kind="ExternalOutput")

    with tile.TileContext(nc) as tc:
        _body(tc, q_d, c_d, kb_d, qm_d, id_d, on_d, out_d, sc_d)
    nc.compile()
    return nc


def _body(tc, q_d, c_d, kb_d, qm_d, id_d, on_d, out_d, sc_d):
    nc = tc.nc
    from contextlib import ExitStack

    ctx = ExitStack()
    with ctx:
        const = ctx.enter_context(tc.tile_pool(name="const", bufs=1))
        qpool = ctx.enter_context(tc.tile_pool(name="q", bufs=2))
        cpool = ctx.enter_context(tc.tile_pool(name="c", bufs=2))
        tpool = ctx.enter_context(tc.tile_pool(name="t", bufs=2))
        sgpool = ctx.enter_context(tc.tile_pool(name="sg", bufs=2))
        mpool = ctx.enter_context(tc.tile_pool(name="m", bufs=2))
        spool = ctx.enter_context(tc.tile_pool(name="s", bufs=2))
        opool = ctx.enter_context(tc.tile_pool(name="o", bufs=3))
        wpool = ctx.enter_context(tc.tile_pool(name="w", bufs=4))
        ps1 = ctx.enter_context(tc.tile_pool(name="ps1", bufs=2, space="PSUM"))
        pst = ctx.enter_context(tc.tile_pool(name="pst", bufs=2, space="PSUM"))
        ps2 = ctx.enter_context(tc.tile_pool(name="ps2", bufs=2, space="PSUM"))
        psd = ctx.enter_context(tc.tile_pool(name="psd", bufs=2, space="PSUM"))

        identb = const.tile([P, P], BF16, tag="identb")
        onesb = const.tile([P, 2], BF16, tag="onesb")
        nc.sync.dma_start(identb[:], id_d[:])
        nc.sync.dma_start(onesb[:], on_d[:])

        st = {}  # per-batch live tiles

        def emit_loads(b):
            kb = mpool.tile([P, NT], F32, tag="kb")
            qm = mpool.tile([P, NT], F32, tag="qm")
            nc.sync.dma_start(kb[:], kb_d[b])
            nc.sync.dma_start(qm[:], qm_d[b])
            qb = qpool.tile([P, NT, D], BF16, tag="qb")
            cb = cpool.tile([P, NT, D], BF16, tag="cb")
            for t in range(NT):
                sl = slice(t * P, (t + 1) * P)
                nc.gpsimd.dma_start(qb[:, t], q_d[b, sl])
                nc.gpsimd.dma_start(cb[:, t], c_d[b, sl])
            st[b] = {"kb": kb, "qm": qm, "qb": qb, "cb": cb}

        def emit_loop_a(b):
            # squares (DVE) + context transposes -> fp8 cT8 (x8 prescale)
            # ssq/inv layout: col 2t = q-tile t, col 2t+1 = c-tile t
            s = st[b]
            ssq = mpool.tile([P, 2 * NT], F32, tag="ssq")
            cT8 = tpool.tile([P, ND, S], FP8, tag="cT8")
            s["ssq"], s["cT8"] = ssq, cT8
            for t in range(NT):
                scr = spool.tile([P, 2, D], BF16, tag="scr")
                nc.vector.tensor_mul(scr[:, 0], s["qb"][:, t], s["qb"][:, t])
                nc.vector.tensor_mul(scr[:, 1], s["cb"][:, t], s["cb"][:, t])
                nc.vector.reduce_sum(
                    ssq[:, 2 * t : 2 * t + 2], scr[:], axis=AX.X
                )
            for t0 in range(0, NT, 2):
                ptc = pst.tile([P, ND, 2, P], BF16, tag="pt")
                for tt in range(2):
                    for dch in range(ND):
                        nc.tensor.transpose(
                            ptc[:, dch, tt],
                            s["cb"][:, t0 + tt, dch * P : (dch + 1) * P],
                            identb[:],
                        )
                nc.scalar.activation(
                    cT8[:, :, t0 * P : (t0 + 2) * P], ptc[:],
                    AF.Copy, scale=8.0,
                )

        def emit_norms(b):
            s = st[b]
            nrm = mpool.tile([P, 2 * NT], F32, tag="nrm")
            inv = mpool.tile([P, 2 * NT], F32, tag="inv")
            nc.scalar.activation(nrm[:], s["ssq"][:], AF.Sqrt)
            nc.vector.reciprocal(inv[:], nrm[:])
            # mm1 runs on fp8 inputs prescaled by 8 (q and c) -> /64 here
            cinv = inv[:].rearrange("p (t two) -> p t two", two=2)
            nc.vector.tensor_scalar_mul(
                cinv[:, :, 1], cinv[:, :, 1], 1.0 / 64.0
            )
            s["inv"] = inv

        def emit_loop_b(b):
            # qn in place (bf16), casting store, q transposes -> fp8 qT8
            s = st[b]
            qT8 = tpool.tile([P, ND, S], FP8, tag="qT8")
            s["qT8"] = qT8
            inv = s["inv"]
            for t in range(NT):
                sl = slice(t * P, (t + 1) * P)
                nc.vector.tensor_scalar_mul(
                    s["qb"][:, t], s["qb"][:, t], inv[:, 2 * t : 2 * t + 1]
                )
                nc.gpsimd.dma_start(out_d[b, sl, 0:D], s["qb"][:, t])
            for t0 in range(0, NT, 2):
                ptq = pst.tile([P, ND, 2, P], BF16, tag="pt")
                for tt in range(2):
                    for dch in range(ND):
                        nc.tensor.transpose(
                            ptq[:, dch, tt],
                            s["qb"][:, t0 + tt, dch * P : (dch + 1) * P],
                            identb[:],
                        )
                nc.scalar.activation(
                    qT8[:, :, t0 * P : (t0 + 2) * P], ptq[:],
                    AF.Copy, scale=8.0,
                )

        def emit_mm1_slot(b, kt):
            # sigT[k, q-halves] for one kt: fp8 DoubleRow, sigmoid evict
            s = st[b]
            acc0 = ps1.tile([P, 512], F32, tag="acc")
            acc1 = ps1.tile([P, 512], F32, tag="acc")
            acc = [acc0, acc1]
            for dg in range(ND // 2):
                for qc in range(2):
                    nc.tensor.matmul(
                        acc[qc][:],
                        s["cT8"][:, 2 * dg : 2 * dg + 2, kt * P : (kt + 1) * P],
                        s["qT8"][:, 2 * dg : 2 * dg + 2, qc * 512 : (qc + 1) * 512],
                        start=(dg == 0),
                        stop=(dg == ND // 2 - 1),
                        perf_mode=PM.DoubleRow,
                    )
            for qc in range(2):
                nc.scalar.activation(
                    s["sg"][:, kt, qc * 512 : (qc + 1) * 512], acc[qc][:],
                    AF.Sigmoid, bias=s["kb"][:, kt : kt + 1],
                    scale=s["inv"][:, 2 * kt + 1 : 2 * kt + 2],
                )

        def emit_mm2_slot(b, qb_i):
            # attended + denominator + scores out for one q block
            s = st[b]
            sg, cb, qm = s["sg"], s["cb"], s["qm"]
            sl = slice(qb_i * P, (qb_i + 1) * P)
            att = ps2.tile([P, 512], F32, tag="att")
            dn = psd.tile([P, 2], F32, tag="dn")
            for kt in range(NT):
                sgblk = sg[:, kt, sl]
                nc.tensor.matmul(
                    att[:], sgblk, cb[:, kt],
                    start=(kt == 0), stop=(kt == NT - 1),
                )
                nc.tensor.matmul(
                    dn[:], sgblk, onesb[:],
                    start=(kt == 0), stop=(kt == NT - 1),
                )
            w = wpool.tile([P, 1], F32, tag="w")
            nc.vector.tensor_scalar_max(w[:], dn[:, 0:1], 1.0)
            nc.vector.reciprocal(w[:], w[:])
            nc.vector.tensor_mul(w[:], w[:], qm[:, qb_i : qb_i + 1])

            ao = opool.tile([P, D], F32, tag="ao")
            nc.vector.tensor_scalar_mul(ao[:], att[:], w[:])
            nc.sync.dma_start(out_d[b, sl, D : 2 * D], ao[:])

            so = opool.tile([P, S], F32, tag="so")
            ptg = pst.tile([P, NT, P], BF16, tag="pt")
            for kt in range(NT):
                nc.tensor.transpose(ptg[:, kt], sg[:, kt, sl], identb[:])
            if qb_i % 2 == 0:
                nc.scalar.activation(so[:], ptg[:], AF.Copy, scale=w[:])
            else:
                nc.vector.tensor_scalar_mul(so[:], ptg[:], w[:])
            nc.sync.dma_start(sc_d[b, sl, :], so[:])

        # ---- pipelined schedule ----
        emit_loads(0)
        emit_loop_a(0)
        for b in range(BPC):
            if b + 1 < BPC:
                emit_loads(b + 1)
            emit_norms(b)
            emit_loop_b(b)
            sg_tile = sgpool.tile([P, NT, S], BF16, tag="sg")
            st[b]["sg"] = sg_tile
            # interleave: mm1 of b with mm2+scores of b-1 (keeps PE fed
            # while ACT drains the sigmoid evictions)
            for i in range(NT):
                emit_mm1_slot(b, i)
                if b - 1 >= 0:
                    emit_mm2_slot(b - 1, i)
            if b - 1 >= 0:
                del st[b - 1]
            if b + 1 < BPC:
                emit_loop_a(b + 1)
        for i in range(NT):
            emit_mm2_slot(BPC - 1, i)


_NC_CACHE = {}


def _get_nc():
    if "nc" not in _NC_CACHE:
        _NC_CACHE["nc"] = build_kernel()
    return _NC_CACHE["nc"]


def _host_inputs(context, query, length):
    iot = np.arange(S)
    keymask = iot[None, :] < length[:, None]                      # [B, S]
    kbH = np.where(keymask, np.float32(0.0), NEG).astype(np.float32)
    kbH = np.ascontiguousarray(kbH.reshape(B, NT, P).transpose(0, 2, 1))
    qmH = keymask.astype(np.float32)
    qmH = np.ascontiguousarray(qmH.reshape(B, NT, P).transpose(0, 2, 1))
    identb = np.eye(P, dtype=ml_dtypes.bfloat16)
    onesb = np.ones((P, 2), dtype=ml_dtypes.bfloat16)
    return kbH, qmH, identb, onesb


def kernel(context, query, length):
    context = np.ascontiguousarray(np.asarray(context, dtype=np.float32))
    query = np.ascontiguousarray(np.asarray(query, dtype=np.float32))
    length = np.asarray(length).astype(np.int64)

    kbH, qmH, identb, onesb = _host_inputs(context, query, length)

    in_maps = []
    for c in range(NCORES):
        sl = slice(c * BPC, (c + 1) * BPC)
        in_maps.append(
            {
                "query": np.ascontiguousarray(query[sl]),
                "context": np.ascontiguousarray(context[sl]),
                "keybias": np.ascontiguousarray(kbH[sl]),
                "qmask": np.ascontiguousarray(qmH[sl]),
                "identb": identb,
                "onesb": onesb,
            }
        )

    nc = _get_nc()
    res = run_bass_kernel_spmd(nc, in_maps, list(range(NCORES)))
    _NC_CACHE["last_result"] = res
    out = np.concatenate([res.results[c]["out"] for c in range(NCORES)], axis=0)
    scores = np.concatenate(
        [res.results[c]["scores"] for c in range(NCORES)], axis=0
    )
    return out, scores


# revision 15
# speedup vs baseline: 1.0869x; 1.0869x over previous
"""Trainium2 Bass kernel for BiLinearSigmoidAttention.

Reference math (per batch b, with L = length[b]):
    qn = l2norm(query), cn = l2norm(context)
    raw[q,k] = qn[q] . cn[k]            (masked: k >= L -> -1e30)
    sig = sigmoid(raw)
    den[q] = max(sum_k sig[q,k], 1)
    scores[q,k] = sig[q,k] / den[q]     (rows q >= L zeroed)
    att[q,:] = sum_k scores[q,k] * context[k,:]
    out = concat([qn, att], -1)
returns (out [B,S,2D], scores [B,S,S])

Device mapping (8 NeuronCores, pure data parallel over B=32 -> 4 per core).

Engine plan per batch (PE kept dense; the whole PE path is bf16 since
walrus forbids mixing 32-bit with 16-bit matmul operands; rel-err budget
is 2e-2, bf16 lands ~3e-3):
  - q/context are loaded straight into bf16 via SWDGE casting DMAs (no
    fp32 staging in SBUF, no engine cast passes).
  - context transposes start as soon as each s-tile lands; q is
    normalized in place (qb *= 1/||q||) after a single batched
    sqrt/reciprocal, stored to out via a casting DMA, then transposed.
  - mm1: sigT[k,q] = sigmoid(cTb.T @ qTb + keybias), weights reused
    across the two q-halves (kt->dch->qc loop order); context l2-norm
    folded into the sigmoid per-partition scale; evicts to bf16 sg.
  - mm2: att[q,d] = sgblk.T @ cb; denominator rides the same weights
    via a tiny ones matmul.
  - scores out: PE transposes of bf16 sg blocks, scaled by w=qmask/den
    during PSUM->SBUF eviction (split across ACT and DVE).
"""

import numpy as np
import ml_dtypes

import concourse.bacc as bacc
import concourse.mybir as mybir
import concourse.tile as tile
from concourse.bass_utils import run_bass_kernel_spmd

B, S, D = 32, 1024, 512
NCORES = 8
BPC = B // NCORES          # batches per core
P = 128                    # partitions
NT = S // P                # 8 s-tiles
ND = D // P                # 4 d-chunks
NEG = np.float32(-1e30)

F32 = mybir.dt.float32
F32R = mybir.dt.float32r
BF16 = mybir.dt.bfloat16
FP8 = mybir.dt.float8e4
PM = mybir.MatmulPerfMode
AF = mybir.ActivationFunctionType
ALU = mybir.AluOpType
AX = mybir.AxisListType


def build_kernel():
    nc = bacc.Bacc("TRN2", target_bir_lowering=False, debug=False)

    q_d = nc.dram_tensor("query", [BPC, S, D], F32, kind="ExternalInput")
    c_d = nc.dram_tensor("context", [BPC, S, D], F32, kind="ExternalInput")
    # keybias[b, p, kt] = 0 if kt*P+p < L else -1e30
    kb_d = nc.dram_tensor("keybias", [BPC, P, NT], F32, kind="ExternalInput")
    # qmask[b, p, qb] = 1 if qb*P+p < L else 0
    qm_d = nc.dram_tensor("qmask", [BPC, P, NT], F32, kind="ExternalInput")
    id_d = nc.dram_tensor("identb", [P, P], BF16, kind="ExternalInput")
    on_d = nc.dram_tensor("onesb", [P, 2], BF16, kind="ExternalInput")
    out_d = nc.dram_tensor("out", [BPC, S, 2 * D], F32, kind="ExternalOutput")
    sc_d = nc.dram_tensor("scores", [BPC, S, S], F32, kind="ExternalOutput")

    with tile.TileContext(nc) as tc:
        _body(tc, q_d, c_d, kb_d, qm_d, id_d, on_d, out_d, sc_d)
    nc.compile()
    return nc


def _body(tc, q_d, c_d, kb_d, qm_d, id_d, on_d, out_d, sc_d):
    nc = tc.nc
    from contextlib import ExitStack

    ctx = ExitStack()
    with ctx:
        const = ctx.enter_context(tc.tile_pool(name="const", bufs=1))
        qpool = ctx.enter_context(tc.tile_pool(name="q", bufs=2))
        cpool = ctx.enter_context(tc.tile_pool(name="c", bufs=3))
        tpool = ctx.enter_context(tc.tile_pool(name="t", bufs=2))
        sgpool = ctx.enter_context(tc.tile_pool(name="sg", bufs=2))
        mpool = ctx.enter_context(tc.tile_pool(name="m", bufs=2))
        spool = ctx.enter_context(tc.tile_pool(name="s", bufs=2))
        opool = ctx.enter_context(tc.tile_pool(name="o", bufs=3))
        wpool = ctx.enter_context(tc.tile_pool(name="w", bufs=4))
        ps1 = ctx.enter_context(tc.tile_pool(name="ps1", bufs=2, space="PSUM"))
        pst = ctx.enter_context(tc.tile_pool(name="pst", bufs=2, space="PSUM"))
        ps2 = ctx.enter_context(tc.tile_pool(name="ps2", bufs=2, space="PSUM"))
        psd = ctx.enter_context(tc.tile_pool(name="psd", bufs=2, space="PSUM"))

        identb = const.tile([P, P], BF16, tag="identb")
        onesb = const.tile([P, 2], BF16, tag="onesb")
        nc.sync.dma_start(identb[:], id_d[:])
        nc.sync.dma_start(onesb[:], on_d[:])

        st = {}  # per-batch live tiles

        def emit_loads(b):
            kb = mpool.tile([P, NT], F32, tag="kb")
            qm = mpool.tile([P, NT], F32, tag="qm")
            nc.sync.dma_start(kb[:], kb_d[b])
            nc.sync.dma_start(qm[:], qm_d[b])
            qb = qpool.tile([P, NT, D], BF16, tag="qb")
            cb = cpool.tile([P, NT, D], BF16, tag="cb")
            for t in range(NT):
                sl = slice(t * P, (t + 1) * P)
                nc.gpsimd.dma_start(qb[:, t], q_d[b, sl])
            for t in range(NT):
                sl = slice(t * P, (t + 1) * P)
                nc.gpsimd.dma_start(cb[:, t], c_d[b, sl])
            ssq = mpool.tile([P, 2 * NT], F32, tag="ssq")
            st[b] = {"kb": kb, "qm": qm, "qb": qb, "cb": cb, "ssq": ssq}

        def emit_squares_pair(b, t0, which):
            # ssq layout: cols 0..7 = q tiles, cols 8..15 = c tiles
            s = st[b]
            src_t = s["qb"] if which == "q" else s["cb"]
            col = t0 if which == "q" else NT + t0
            scr = spool.tile([P, 2, D], BF16, tag="scr")
            nc.vector.tensor_mul(scr[:, 0], src_t[:, t0], src_t[:, t0])
            nc.vector.tensor_mul(scr[:, 1], src_t[:, t0 + 1], src_t[:, t0 + 1])
            nc.vector.reduce_sum(
                s["ssq"][:, col : col + 2], scr[:], axis=AX.X
            )

        def emit_norms(b):
            s = st[b]
            nrm = mpool.tile([P, 2 * NT], F32, tag="nrm")
            inv = mpool.tile([P, 2 * NT], F32, tag="inv")
            nc.scalar.activation(nrm[:], s["ssq"][:], AF.Sqrt)
            nc.vector.reciprocal(inv[:], nrm[:])
            # mm1 runs on fp8 inputs prescaled by 8 (q and c) -> /64 here
            nc.vector.tensor_scalar_mul(
                inv[:, NT : 2 * NT], inv[:, NT : 2 * NT], 1.0 / 64.0
            )
            s["inv"] = inv
            qT8 = tpool.tile([P, ND, S], FP8, tag="qT8")
            cT8 = tpool.tile([P, ND, S], FP8, tag="cT8")
            s["qT8"], s["cT8"] = qT8, cT8

        def emit_qtrans_pair(b, t0):
            # qn in place (bf16), casting store, q transposes -> fp8 qT8
            s = st[b]
            for tt in range(2):
                t = t0 + tt
                sl = slice(t * P, (t + 1) * P)
                nc.vector.tensor_scalar_mul(
                    s["qb"][:, t], s["qb"][:, t], s["inv"][:, t : t + 1]
                )
                nc.gpsimd.dma_start(out_d[b, sl, 0:D], s["qb"][:, t])
            ptq = pst.tile([P, ND, 2, P], BF16, tag="pt")
            for tt in range(2):
                for dch in range(ND):
                    nc.tensor.transpose(
                        ptq[:, dch, tt],
                        s["qb"][:, t0 + tt, dch * P : (dch + 1) * P],
                        identb[:],
                    )
            nc.vector.tensor_scalar_mul(
                s["qT8"][:, :, t0 * P : (t0 + 2) * P], ptq[:], 8.0
            )

        def emit_ctrans_pair(b, t0):
            s = st[b]
            ptc = pst.tile([P, ND, 2, P], BF16, tag="pt")
            for tt in range(2):
                for dch in range(ND):
                    nc.tensor.transpose(
                        ptc[:, dch, tt],
                        s["cb"][:, t0 + tt, dch * P : (dch + 1) * P],
                        identb[:],
                    )
            nc.scalar.activation(
                s["cT8"][:, :, t0 * P : (t0 + 2) * P], ptc[:],
                AF.Copy, scale=8.0,
            )

        def emit_mm1_slot(b, kt):
            # sigT[k, q-halves] for one kt: fp8 DoubleRow, sigmoid evict
            s = st[b]
            acc0 = ps1.tile([P, 512], F32, tag="acc")
            acc1 = ps1.tile([P, 512], F32, tag="acc")
            acc = [acc0, acc1]
            for dg in range(ND // 2):
                for qc in range(2):
                    nc.tensor.matmul(
                        acc[qc][:],
                        s["cT8"][:, 2 * dg : 2 * dg + 2, kt * P : (kt + 1) * P],
                        s["qT8"][:, 2 * dg : 2 * dg + 2, qc * 512 : (qc + 1) * 512],
                        start=(dg == 0),
                        stop=(dg == ND // 2 - 1),
                        perf_mode=PM.DoubleRow,
                    )
            for qc in range(2):
                nc.scalar.activation(
                    s["sg"][:, kt, qc * 512 : (qc + 1) * 512], acc[qc][:],
                    AF.Sigmoid, bias=s["kb"][:, kt : kt + 1],
                    scale=s["inv"][:, NT + kt : NT + kt + 1],
                )

        def emit_mm2_slot(b, qb_i):
            # attended + denominator + scores out for one q block
            s = st[b]
            sg, cb, qm = s["sg"], s["cb"], s["qm"]
            sl = slice(qb_i * P, (qb_i + 1) * P)
            att = ps2.tile([P, 512], F32, tag="att")
            dn = psd.tile([P, 2], F32, tag="dn")
            for kt in range(NT):
                sgblk = sg[:, kt, sl]
                nc.tensor.matmul(
                    att[:], sgblk, cb[:, kt],
                    start=(kt == 0), stop=(kt == NT - 1),
                )
                nc.tensor.matmul(
                    dn[:], sgblk, onesb[:],
                    start=(kt == 0), stop=(kt == NT - 1),
                )
            w = wpool.tile([P, 1], F32, tag="w")
            nc.vector.tensor_scalar_max(w[:], dn[:, 0:1], 1.0)
            nc.vector.reciprocal(w[:], w[:])
            nc.vector.tensor_mul(w[:], w[:], qm[:, qb_i : qb_i + 1])

            ao = opool.tile([P, D], F32, tag="ao")
            nc.vector.tensor_scalar_mul(ao[:], att[:], w[:])
            nc.sync.dma_start(out_d[b, sl, D : 2 * D], ao[:])

            so = opool.tile([P, S], F32, tag="so")
            ptg = pst.tile([P, NT, P], BF16, tag="pt")
            for kt in range(NT):
                nc.tensor.transpose(ptg[:, kt], sg[:, kt, sl], identb[:])
            if qb_i % 2 == 0:
                nc.scalar.activation(so[:], ptg[:], AF.Copy, scale=w[:])
            else:
                nc.vector.tensor_scalar_mul(so[:], ptg[:], w[:])
            nc.sync.dma_start(sc_d[b, sl, :], so[:])

        # ---- 3-stage pipelined schedule ----
        # iter b: slots run mm1(b) + mm2(b-1); squares(b+1) ride the
        # slot tail; norms(b+1) + transposes(b+1) close the iteration
        # interleaved q/c so eviction engines (ACT for c, DVE for q)
        # alternate.
        def emit_prologue():
            emit_loads(0)
            for t0 in range(0, NT, 2):
                emit_squares_pair(0, t0, "q")
            for t0 in range(0, NT, 2):
                emit_squares_pair(0, t0, "c")
            emit_norms(0)
            for t0 in range(0, NT, 2):
                emit_qtrans_pair(0, t0)
                emit_ctrans_pair(0, t0)

        emit_prologue()
        for b in range(BPC):
            if b + 1 < BPC:
                emit_loads(b + 1)
            sg_tile = sgpool.tile([P, NT, S], BF16, tag="sg")
            st[b]["sg"] = sg_tile
            for i in range(NT):
                emit_mm1_slot(b, i)
                if b - 1 >= 0:
                    emit_mm2_slot(b - 1, i)
                if b + 1 < BPC:
                    if i >= 2 and i <= 5:            # q squares: slots 2-5
                        emit_squares_pair(b + 1, 2 * (i - 2), "q")
                    elif i >= 6:                     # c squares: slots 6-7
                        emit_squares_pair(b + 1, 4 * (i - 6), "c")
                        emit_squares_pair(b + 1, 4 * (i - 6) + 2, "c")
            if b - 1 >= 0:
                del st[b - 1]
            if b + 1 < BPC:
                emit_norms(b + 1)
                for t0 in range(0, NT, 2):
                    emit_qtrans_pair(b + 1, t0)
                    emit_ctrans_pair(b + 1, t0)
        for i in range(NT):
            emit_mm2_slot(BPC - 1, i)


_NC_CACHE = {}


def _get_nc():
    if "nc" not in _NC_CACHE:
        _NC_CACHE["nc"] = build_kernel()
    return _NC_CACHE["nc"]


def _host_inputs(context, query, length):
    iot = np.arange(S)
    keymask = iot[None, :] < length[:, None]                      # [B, S]
    kbH = np.where(keymask, np.float32(0.0), NEG).astype(np.float32)
    kbH = np.ascontiguousarray(kbH.reshape(B, NT, P).transpose(0, 2, 1))
    qmH = keymask.astype(np.float32)
    qmH = np.ascontiguousarray(qmH.reshape(B, NT, P).transpose(0, 2, 1))
    identb = np.eye(P, dtype=ml_dtypes.bfloat16)
    onesb = np.ones((P, 2), dtype=ml_dtypes.bfloat16)
    return kbH, qmH, identb, onesb


def kernel(context, query, length):
    context = np.ascontiguousarray(np.asarray(context, dtype=np.float32))
    query = np.ascontiguousarray(np.asarray(query, dtype=np.float32))
    length = np.asarray(length).astype(np.int64)

    kbH, qmH, identb, onesb = _host_inputs(context, query, length)

    in_maps = []
    for c in range(NCORES):
        sl = slice(c * BPC, (c + 1) * BPC)
        in_maps.append(
            {
                "query": np.ascontiguousarray(query[sl]),
                "context": np.ascontiguousarray(context[sl]),
                "keybias": np.ascontiguousarray(kbH[sl]),
                "qmask": np.ascontiguousarray(qmH[sl]),
                "identb": identb,
                "onesb": onesb,
            }
        )

    nc = _get_nc()
    res = run_bass_kernel_spmd(nc, in_maps, list(range(NCORES)))
    _NC_CACHE["last_result"] = res
    out = np.concatenate([res.results[c]["out"] for c in range(NCORES)], axis=0)
    scores = np.concatenate(
        [res.results[c]["scores"] for c in range(NCORES)], axis=0
    )
    return out, scores


# revision 16
# speedup vs baseline: 1.1512x; 1.0591x over previous
"""Trainium2 Bass kernel for BiLinearSigmoidAttention.

Reference math (per batch b, with L = length[b]):
    qn = l2norm(query), cn = l2norm(context)
    raw[q,k] = qn[q] . cn[k]            (masked: k >= L -> -1e30)
    sig = sigmoid(raw)
    den[q] = max(sum_k sig[q,k], 1)
    scores[q,k] = sig[q,k] / den[q]     (rows q >= L zeroed)
    att[q,:] = sum_k scores[q,k] * context[k,:]
    out = concat([qn, att], -1)
returns (out [B,S,2D], scores [B,S,S])

Device mapping (8 NeuronCores, pure data parallel over B=32 -> 4 per core).

Engine plan per batch (PE kept dense; the whole PE path is bf16 since
walrus forbids mixing 32-bit with 16-bit matmul operands; rel-err budget
is 2e-2, bf16 lands ~3e-3):
  - q/context are loaded straight into bf16 via SWDGE casting DMAs (no
    fp32 staging in SBUF, no engine cast passes).
  - context transposes start as soon as each s-tile lands; q is
    normalized in place (qb *= 1/||q||) after a single batched
    sqrt/reciprocal, stored to out via a casting DMA, then transposed.
  - mm1: sigT[k,q] = sigmoid(cTb.T @ qTb + keybias), weights reused
    across the two q-halves (kt->dch->qc loop order); context l2-norm
    folded into the sigmoid per-partition scale; evicts to bf16 sg.
  - mm2: att[q,d] = sgblk.T @ cb; denominator rides the same weights
    via a tiny ones matmul.
  - scores out: PE transposes of bf16 sg blocks, scaled by w=qmask/den
    during PSUM->SBUF eviction (split across ACT and DVE).
"""

import numpy as np
import ml_dtypes

import concourse.bacc as bacc
import concourse.mybir as mybir
import concourse.tile as tile
from concourse.bass_utils import run_bass_kernel_spmd

B, S, D = 32, 1024, 512
NCORES = 8
BPC = B // NCORES          # batches per core
P = 128                    # partitions
NT = S // P                # 8 s-tiles
ND = D // P                # 4 d-chunks
NEG = np.float32(-1e30)

F32 = mybir.dt.float32
F32R = mybir.dt.float32r
BF16 = mybir.dt.bfloat16
FP8 = mybir.dt.float8e4
PM = mybir.MatmulPerfMode
AF = mybir.ActivationFunctionType
ALU = mybir.AluOpType
AX = mybir.AxisListType


def build_kernel():
    nc = bacc.Bacc("TRN2", target_bir_lowering=False, debug=False)

    q_d = nc.dram_tensor("query", [BPC, S, D], F32, kind="ExternalInput")
    c_d = nc.dram_tensor("context", [BPC, S, D], F32, kind="ExternalInput")
    # keybias[b, p, kt] = 0 if kt*P+p < L else -1e30
    kb_d = nc.dram_tensor("keybias", [BPC, P, NT], F32, kind="ExternalInput")
    # qmask[b, p, qb] = 1 if qb*P+p < L else 0
    qm_d = nc.dram_tensor("qmask", [BPC, P, NT], F32, kind="ExternalInput")
    id_d = nc.dram_tensor("identb", [P, P], BF16, kind="ExternalInput")
    on_d = nc.dram_tensor("onesb", [P, 2], BF16, kind="ExternalInput")
    out_d = nc.dram_tensor("out", [BPC, S, 2 * D], F32, kind="ExternalOutput")
    sc_d = nc.dram_tensor("scores", [BPC, S, S], F32, kind="ExternalOutput")

    with tile.TileContext(nc) as tc:
        _body(tc, q_d, c_d, kb_d, qm_d, id_d, on_d, out_d, sc_d)
    nc.compile()
    return nc


def _body(tc, q_d, c_d, kb_d, qm_d, id_d, on_d, out_d, sc_d):
    nc = tc.nc
    from contextlib import ExitStack

    ctx = ExitStack()
    with ctx:
        const = ctx.enter_context(tc.tile_pool(name="const", bufs=1))
        qpool = ctx.enter_context(tc.tile_pool(name="q", bufs=2))
        cpool = ctx.enter_context(tc.tile_pool(name="c", bufs=3))
        tpool = ctx.enter_context(tc.tile_pool(name="t", bufs=2))
        sgpool = ctx.enter_context(tc.tile_pool(name="sg", bufs=2))
        mpool = ctx.enter_context(tc.tile_pool(name="m", bufs=2))
        spool = ctx.enter_context(tc.tile_pool(name="s", bufs=2))
        opool = ctx.enter_context(tc.tile_pool(name="o", bufs=3))
        wpool = ctx.enter_context(tc.tile_pool(name="w", bufs=4))
        ps1 = ctx.enter_context(tc.tile_pool(name="ps1", bufs=2, space="PSUM"))
        pst = ctx.enter_context(tc.tile_pool(name="pst", bufs=2, space="PSUM"))
        ps2 = ctx.enter_context(tc.tile_pool(name="ps2", bufs=2, space="PSUM"))
        psd = ctx.enter_context(tc.tile_pool(name="psd", bufs=2, space="PSUM"))

        identb = const.tile([P, P], BF16, tag="identb")
        onesb = const.tile([P, 2], BF16, tag="onesb")
        nc.sync.dma_start(identb[:], id_d[:])
        nc.sync.dma_start(onesb[:], on_d[:])

        st = {}  # per-batch live tiles

        def emit_loads(b):
            kb = mpool.tile([P, NT], F32, tag="kb")
            qm = mpool.tile([P, NT], F32, tag="qm")
            nc.sync.dma_start(kb[:], kb_d[b])
            nc.sync.dma_start(qm[:], qm_d[b])
            qb = qpool.tile([P, NT, D], BF16, tag="qb")
            cb = cpool.tile([P, NT, D], BF16, tag="cb")
            for t in range(NT):
                sl = slice(t * P, (t + 1) * P)
                nc.gpsimd.dma_start(cb[:, t], c_d[b, sl])
            for t in range(NT):
                sl = slice(t * P, (t + 1) * P)
                nc.gpsimd.dma_start(qb[:, t], q_d[b, sl])
            ssq = mpool.tile([P, 2 * NT], F32, tag="ssq")
            st[b] = {"kb": kb, "qm": qm, "qb": qb, "cb": cb, "ssq": ssq}

        def emit_squares_pair(b, t0, which):
            # ssq layout: cols 0..7 = q tiles, cols 8..15 = c tiles
            s = st[b]
            src_t = s["qb"] if which == "q" else s["cb"]
            col = t0 if which == "q" else NT + t0
            scr = spool.tile([P, 2, D], BF16, tag="scr")
            nc.vector.tensor_mul(scr[:, 0], src_t[:, t0], src_t[:, t0])
            nc.vector.tensor_mul(scr[:, 1], src_t[:, t0 + 1], src_t[:, t0 + 1])
            nc.vector.reduce_sum(
                s["ssq"][:, col : col + 2], scr[:], axis=AX.X
            )

        def emit_norms(b, alloc=True):
            s = st[b]
            nrm = mpool.tile([P, 2 * NT], F32, tag="nrm")
            inv = mpool.tile([P, 2 * NT], F32, tag="inv")
            nc.scalar.activation(nrm[:], s["ssq"][:], AF.Sqrt)
            nc.vector.reciprocal(inv[:], nrm[:])
            # mm1 runs on fp8 inputs prescaled by 8 (q and c) -> /64 here
            nc.vector.tensor_scalar_mul(
                inv[:, NT : 2 * NT], inv[:, NT : 2 * NT], 1.0 / 64.0
            )
            s["inv"] = inv
            if alloc:
                qT8 = tpool.tile([P, ND, S], FP8, tag="qT8")
                cT8 = tpool.tile([P, ND, S], FP8, tag="cT8")
                s["qT8"], s["cT8"] = qT8, cT8

        def emit_qtrans_pair(b, t0):
            # qn in place (bf16), casting store, q transposes -> fp8 qT8
            s = st[b]
            for tt in range(2):
                t = t0 + tt
                sl = slice(t * P, (t + 1) * P)
                nc.vector.tensor_scalar_mul(
                    s["qb"][:, t], s["qb"][:, t], s["inv"][:, t : t + 1]
                )
                nc.gpsimd.dma_start(out_d[b, sl, 0:D], s["qb"][:, t])
            ptq = pst.tile([P, ND, 2, P], BF16, tag="pt")
            for tt in range(2):
                for dch in range(ND):
                    nc.tensor.transpose(
                        ptq[:, dch, tt],
                        s["qb"][:, t0 + tt, dch * P : (dch + 1) * P],
                        identb[:],
                    )
            nc.scalar.activation(
                s["qT8"][:, :, t0 * P : (t0 + 2) * P], ptq[:],
                AF.Copy, scale=8.0,
            )

        def emit_ctrans_pair(b, t0):
            s = st[b]
            ptc = pst.tile([P, ND, 2, P], BF16, tag="pt")
            for tt in range(2):
                for dch in range(ND):
                    nc.tensor.transpose(
                        ptc[:, dch, tt],
                        s["cb"][:, t0 + tt, dch * P : (dch + 1) * P],
                        identb[:],
                    )
            nc.scalar.activation(
                s["cT8"][:, :, t0 * P : (t0 + 2) * P], ptc[:],
                AF.Copy, scale=8.0,
            )

        def emit_mm1_slot(b, kt):
            # sigT[k, q-halves] for one kt: fp8 DoubleRow, sigmoid evict
            s = st[b]
            acc0 = ps1.tile([P, 512], F32, tag="acc")
            acc1 = ps1.tile([P, 512], F32, tag="acc")
            acc = [acc0, acc1]
            for dg in range(ND // 2):
                for qc in range(2):
                    nc.tensor.matmul(
                        acc[qc][:],
                        s["cT8"][:, 2 * dg : 2 * dg + 2, kt * P : (kt + 1) * P],
                        s["qT8"][:, 2 * dg : 2 * dg + 2, qc * 512 : (qc + 1) * 512],
                        start=(dg == 0),
                        stop=(dg == ND // 2 - 1),
                        perf_mode=PM.DoubleRow,
                    )
            for qc in range(2):
                nc.scalar.activation(
                    s["sg"][:, kt, qc * 512 : (qc + 1) * 512], acc[qc][:],
                    AF.Sigmoid, bias=s["kb"][:, kt : kt + 1],
                    scale=s["inv"][:, NT + kt : NT + kt + 1],
                )

        def emit_mm2_slot(b, qb_i):
            # attended + denominator + scores out for one q block
            s = st[b]
            sg, cb, qm = s["sg"], s["cb"], s["qm"]
            sl = slice(qb_i * P, (qb_i + 1) * P)
            att = ps2.tile([P, 512], F32, tag="att")
            dn = psd.tile([P, 2], F32, tag="dn")
            for kt in range(NT):
                sgblk = sg[:, kt, sl]
                nc.tensor.matmul(
                    att[:], sgblk, cb[:, kt],
                    start=(kt == 0), stop=(kt == NT - 1),
                )
                nc.tensor.matmul(
                    dn[:], sgblk, onesb[:],
                    start=(kt == 0), stop=(kt == NT - 1),
                )
            w = wpool.tile([P, 1], F32, tag="w")
            nc.vector.tensor_scalar_max(w[:], dn[:, 0:1], 1.0)
            nc.vector.reciprocal(w[:], w[:])
            nc.vector.tensor_mul(w[:], w[:], qm[:, qb_i : qb_i + 1])

            ao = opool.tile([P, D], F32, tag="ao")
            nc.vector.tensor_scalar_mul(ao[:], att[:], w[:])
            nc.sync.dma_start(out_d[b, sl, D : 2 * D], ao[:])

            so = opool.tile([P, S], F32, tag="so")
            ptg = pst.tile([P, NT, P], BF16, tag="pt")
            for kt in range(NT):
                nc.tensor.transpose(ptg[:, kt], sg[:, kt, sl], identb[:])
            if qb_i % 8 in (0, 2, 4, 5, 7):
                nc.scalar.activation(so[:], ptg[:], AF.Copy, scale=w[:])
            else:
                nc.vector.tensor_scalar_mul(so[:], ptg[:], w[:])
            nc.sync.dma_start(sc_d[b, sl, :], so[:])

        # ---- 3-stage pipelined schedule ----
        # iter b: slots run mm1(b) + mm2(b-1); squares(b+1) ride the
        # slot tail; norms(b+1) + transposes(b+1) close the iteration
        # interleaved q/c so eviction engines (ACT for c, DVE for q)
        # alternate.
        def emit_prologue():
            emit_loads(0)
            # cT8 alloc up front so c transposes can evict before norms
            qT8 = tpool.tile([P, ND, S], FP8, tag="qT8")
            cT8 = tpool.tile([P, ND, S], FP8, tag="cT8")
            st[0]["qT8"], st[0]["cT8"] = qT8, cT8
            for t0 in range(0, NT, 2):
                emit_ctrans_pair(0, t0)
                emit_squares_pair(0, t0, "c")
                emit_squares_pair(0, t0, "q")
            emit_norms(0, alloc=False)
            for t0 in range(0, NT, 2):
                emit_qtrans_pair(0, t0)

        emit_prologue()
        for b in range(BPC):
            if b + 1 < BPC:
                emit_loads(b + 1)
            sg_tile = sgpool.tile([P, NT, S], BF16, tag="sg")
            st[b]["sg"] = sg_tile
            for i in range(NT):
                emit_mm1_slot(b, i)
                if b - 1 >= 0:
                    emit_mm2_slot(b - 1, i)
                if b + 1 < BPC:
                    if i >= 2 and i <= 5:            # q squares: slots 2-5
                        emit_squares_pair(b + 1, 2 * (i - 2), "q")
                    elif i >= 6:                     # c squares: slots 6-7
                        emit_squares_pair(b + 1, 4 * (i - 6), "c")
                        emit_squares_pair(b + 1, 4 * (i - 6) + 2, "c")
            if b - 1 >= 0:
                del st[b - 1]
            if b + 1 < BPC:
                emit_norms(b + 1)
                for t0 in range(0, NT, 2):
                    emit_ctrans_pair(b + 1, t0)
                    emit_qtrans_pair(b + 1, t0)
        for i in range(NT):
            emit_mm2_slot(BPC - 1, i)


_NC_CACHE = {}


def _get_nc():
    if "nc" not in _NC_CACHE:
        _NC_CACHE["nc"] = build_kernel()
    return _NC_CACHE["nc"]


def _host_inputs(context, query, length):
    iot = np.arange(S)
    keymask = iot[None, :] < length[:, None]                      # [B, S]
    kbH = np.where(keymask, np.float32(0.0), NEG).astype(np.float32)
    kbH = np.ascontiguousarray(kbH.reshape(B, NT, P).transpose(0, 2, 1))
    qmH = keymask.astype(np.float32)
    qmH = np.ascontiguousarray(qmH.reshape(B, NT, P).transpose(0, 2, 1))
    identb = np.eye(P, dtype=ml_dtypes.bfloat16)
    onesb = np.ones((P, 2), dtype=ml_dtypes.bfloat16)
    return kbH, qmH, identb, onesb


def kernel(context, query, length):
    context = np.ascontiguousarray(np.asarray(context, dtype=np.float32))
    query = np.ascontiguousarray(np.asarray(query, dtype=np.float32))
    length = np.asarray(length).astype(np.int64)

    kbH, qmH, identb, onesb = _host_inputs(context, query, length)

    in_maps = []
    for c in range(NCORES):
        sl = slice(c * BPC, (c + 1) * BPC)
        in_maps.append(
            {
                "query": np.ascontiguousarray(query[sl]),
                "context": np.ascontiguousarray(context[sl]),
                "keybias": np.ascontiguousarray(kbH[sl]),
                "qmask": np.ascontiguousarray(qmH[sl]),
                "identb": identb,
                "onesb": onesb,
            }
        )

    nc = _get_nc()
    res = run_bass_kernel_spmd(nc, in_maps, list(range(NCORES)))
    _NC_CACHE["last_result"] = res
    out = np.concatenate([res.results[c]["out"] for c in range(NCORES)], axis=0)
    scores = np.concatenate(
        [res.results[c]["scores"] for c in range(NCORES)], axis=0
    )
    return out, scores
